# revision 18
# baseline (speedup 1.0000x reference)
"""3-layer GAT (graph attention network) on Trainium2 — Bass/Tile, 8-core SPMD.

v2 redesign vs baseline:
  * Aligned 128-node groups (group g = local nodes [g*128,(g+1)*128)): kills
    scratch-row indirection, the x_own gathers, and the host-side output
    permute.  dstrel = dst % 128.
  * fp8 (e4m3) feature table for layers 0/1: row = [feat*16 fp8 x128 | el
    bf16 xH | pad] = 256B (vs 512B bf16) — halves feat-gather HBM traffic.
    Layer 2 row stays bf16: [feat x64 | el | pad] = 256B.
  * One dma_gather per chunk-zone (few thousand idxs) instead of 1024-idx
    pieces: SWDGE fixed cost is 994ns/call, so baseline burned ~760us of Q7
    on call overhead alone.
  * er gathers issued before feat gathers each chunk so their drain overlaps
    the table AllGather (feat gathers depend on the AG, er ones don't).
  * Denominator folded into the scatter matmul: rhs = [w*feat | w], one
    matmul per edge tile instead of two.
  * Phase A: one matmul per block against [W|Wal|War] concat, bf16.
Edge softmax skips the segment-max subtraction (logits are O(1), exp is
safe): alpha = exp(e)/sum(exp(e)).
"""

import numpy as np

# ---------------- static problem config (self-contained) ---------------------
N_CORES = 8
NEG_SLOPE = 0.2
P = 128
SPAN = 4                   # blocks per gather chunk
SPLIT = 32768              # int16 index split point
# (in_dim, H, D, apply_relu) per layer
LAYERS = [(128, 4, 32, True), (128, 4, 32, True), (128, 1, 64, False)]
OUT_DIM = 64
FP8_SCALE = 16.0           # feat scale before e4m3 cast (layers 0/1)

_cache = {}
last_run_info = {}


# ============================ host-side preprocessing ========================

def _wrap16(vals, cols):
    """dma_gather index layout: entry i -> [i % 16, i // 16], replicated
    across the 8 groups of 16 partitions."""
    t = np.zeros((16, cols), np.int16)
    n = len(vals)
    t[np.arange(n) % 16, np.arange(n) // 16] = vals.astype(np.int16)
    return np.tile(t, (8, 1))


def _preprocess(src, dst, n_nodes, n_cores):
    npc = n_nodes // n_cores
    xj = (npc + P - 1) // P        # number of 128-node blocks (= groups)
    cores = []
    for c in range(n_cores):
        lo = c * npc
        m = (dst >= lo) & (dst < lo + npc)
        s = src[m].astype(np.int64)
        d = (dst[m] - lo).astype(np.int64)
        o = np.argsort(d, kind="stable")
        cores.append((s[o], d[o]))

    # per-block lo/hi edge counts; shared tile counts = max across cores
    TL = np.zeros(xj, np.int64)
    TH = np.zeros(xj, np.int64)
    for (s, d) in cores:
        blk = d // P
        for g in range(xj):
            mg = blk == g
            nlo = int((s[mg] < SPLIT).sum())
            nhi = int(mg.sum()) - nlo
            TL[g] = max(TL[g], (nlo + P - 1) // P)
            TH[g] = max(TH[g], (nhi + P - 1) // P)
    lo_base = np.zeros(xj + 1, np.int64)
    hi_base = np.zeros(xj + 1, np.int64)
    np.cumsum(TL, out=lo_base[1:])
    np.cumsum(TH, out=hi_base[1:])
    SL = int(lo_base[xj]) * P
    SH = int(hi_base[xj]) * P
    SLP = max(SL, 2048)
    SHP = max(SH, 2048)

    per_core = []
    for (s, d) in cores:
        idx_lo = np.zeros(SL, np.int64)
        idx_hi = np.zeros(SH, np.int64)
        er_lo = np.zeros(SL, np.int64)
        er_hi = np.zeros(SH, np.int64)
        dr_lo = np.full(SL, -1.0, np.float32)
        dr_hi = np.full(SH, -1.0, np.float32)
        blk = d // P
        for g in range(xj):
            mg = blk == g
            gs = s[mg]
            gd = d[mg]
            lm = gs < SPLIT
            for zone, msk in ((0, lm), (1, ~lm)):
                zs = gs[msk]
                zd = gd[msk]
                n = len(zd)
                o = (int(lo_base[g]) if zone == 0 else int(hi_base[g])) * P
                ti = idx_lo if zone == 0 else idx_hi
                te = er_lo if zone == 0 else er_hi
                tr = dr_lo if zone == 0 else dr_hi
                ti[o:o + n] = zs - (0 if zone == 0 else SPLIT)
                te[o:o + n] = zd
                tr[o:o + n] = (zd % P).astype(np.float32)

        def _padcols(a, cols):
            out = np.zeros((a.shape[0], cols), a.dtype)
            out[:, :a.shape[1]] = a
            return out

        import ml_dtypes
        bf = ml_dtypes.bfloat16
        dl = dr_lo.reshape(max(SL // P, 1) if SL else 1, P)[:SL // P].T.astype(bf) \
            if SL else np.zeros((P, 0), bf)
        dh = dr_hi.reshape(max(SH // P, 1) if SH else 1, P)[:SH // P].T.astype(bf) \
            if SH else np.zeros((P, 0), bf)
        per_core.append(dict(
            idx_lo=_padcols(_wrap16(idx_lo, max(SL // 16, 1)), SLP // 16),
            idx_hi=_padcols(_wrap16(idx_hi, max(SH // 16, 1)), SHP // 16),
            idx_er_lo=_padcols(_wrap16(er_lo, max(SL // 16, 1)), SLP // 16),
            idx_er_hi=_padcols(_wrap16(er_hi, max(SH // 16, 1)), SHP // 16),
            dr_lo=_padcols(dl, SLP // P),
            dr_hi=_padcols(dh, SHP // P),
        ))
    meta = dict(TL=tuple(int(x) for x in TL), TH=tuple(int(x) for x in TH),
                SL=SL, SH=SH, SLP=SLP, SHP=SHP,
                xj=xj, npc=npc, n_nodes=n_nodes, n_cores=n_cores)
    return meta, per_core


# ============================ device program =================================

def _build_program(meta):
    import concourse.bass as bass
    import concourse.tile as tile
    from concourse import bacc, mybir

    def _midb(ap, n):
        # [P, D] -> [P, n, D] with the middle dim broadcast (step 0)
        return bass.AP(ap.tensor, ap.offset,
                       [list(ap.ap[0]), [0, n], list(ap.ap[1])])

    f32 = mybir.dt.float32
    bf16 = mybir.dt.bfloat16
    fp8 = mybir.dt.float8e4
    i16 = mybir.dt.int16
    AF = mybir.ActivationFunctionType
    OP = mybir.AluOpType

    SL, SH = meta["SL"], meta["SH"]
    SLP, SHP = meta["SLP"], meta["SHP"]
    TL, TH = meta["TL"], meta["TH"]
    xj, npc = meta["xj"], meta["npc"]
    n_nodes, n_cores = meta["n_nodes"], meta["n_cores"]
    NPCP = xj * P
    lo_base = np.concatenate([[0], np.cumsum(TL)]).astype(int)
    hi_base = np.concatenate([[0], np.cumsum(TH)]).astype(int)
    nchunk = (xj + SPAN - 1) // SPAN
    # max tiles in any chunk (for buffer sizing), per zone
    MCL = max(int(lo_base[min(k * SPAN + SPAN, xj)] - lo_base[k * SPAN])
              for k in range(nchunk))
    MCH = max(int(hi_base[min(k * SPAN + SPAN, xj)] - hi_base[k * SPAN])
              for k in range(nchunk))
    MG = max(max(TL), max(TH))   # max tiles per block-zone
    ER_LA = 3                    # er-gather chunk lookahead (erpool bufs)
    FG_LA = 3                    # feat-gather chunk lookahead (fgpool bufs)

    nc = bacc.Bacc("TRN2", target_bir_lowering=False, debug=False,
                   enable_asserts=False, num_devices=n_cores,
                   num_swdge_queues=4)

    _qctr = [0]

    def _gather(out_ap3, in_ap, idxs2, ni, elem, piece=1536):
        """Split into <=piece-idx sub-gathers cycling the 4 SWDGE queues.
        Pieces must fit the per-queue descriptor ring (~4096 descs with the
        default 16KB DynamicDMAScratch); an oversized gather stalls Q7 at
        drain rate and serializes all queues."""
        ntile = ni // P
        pt = max(piece // P, 1)
        for j0 in range(0, ntile, pt):
            j1 = min(j0 + pt, ntile)
            n = (j1 - j0) * P
            q = _qctr[0] % 4
            _qctr[0] += 1
            nc.gpsimd.dma_gather(
                out_ap=out_ap3[:, j0:j1, :], in_ap=in_ap,
                idxs_ap=idxs2[:, j0 * 8:j1 * 8],
                num_idxs=n, num_idxs_reg=n, elem_size=elem,
                single_packet=False, queue_num=q)

    t_feats = nc.dram_tensor("features_own", [NPCP, 128], f32,
                             kind="ExternalInput").ap()
    t_idx_lo = nc.dram_tensor("idx_lo", [P, SLP // 16], i16,
                              kind="ExternalInput").ap()
    t_idx_hi = nc.dram_tensor("idx_hi", [P, SHP // 16], i16,
                              kind="ExternalInput").ap()
    t_ier_lo = nc.dram_tensor("idx_er_lo", [P, SLP // 16], i16,
                              kind="ExternalInput").ap()
    t_ier_hi = nc.dram_tensor("idx_er_hi", [P, SHP // 16], i16,
                              kind="ExternalInput").ap()
    t_dr_lo = nc.dram_tensor("dr_lo", [P, SLP // P], bf16,
                             kind="ExternalInput").ap()
    t_dr_hi = nc.dram_tensor("dr_hi", [P, SHP // P], bf16,
                             kind="ExternalInput").ap()
    t_iota = nc.dram_tensor("iota_rep", [P, P], bf16,
                            kind="ExternalInput").ap()
    t_ident = nc.dram_tensor("identity_bf", [P, P], bf16,
                             kind="ExternalInput").ap()
    t_W, t_b = [], []
    for li, (ind, H, D, _) in enumerate(LAYERS):
        hd = H * D
        t_W.append(nc.dram_tensor(f"Wcat{li}", [ind, hd + 2 * H], bf16,
                                  kind="ExternalInput").ap())
        t_b.append(nc.dram_tensor(f"br{li}", [P, hd], f32,
                                  kind="ExternalInput").ap())
    t_out = nc.dram_tensor("out", [npc, OUT_DIM], f32,
                           kind="ExternalOutput").ap()

    with tile.TileContext(nc) as tc:
        with (
            tc.tile_pool(name="const", bufs=1) as cpool,
            tc.tile_pool(name="big", bufs=1) as bigpool,
            tc.tile_pool(name="sb", bufs=3) as sb,
            tc.tile_pool(name="fg", bufs=3) as fgpool,
            tc.tile_pool(name="er", bufs=3) as erpool,
            tc.tile_pool(name="wp", bufs=3) as wpool,
            tc.tile_pool(name="ps", bufs=3, space="PSUM") as pspool,
            tc.tile_pool(name="psA", bufs=2, space="PSUM") as psA,
            tc.tile_pool(name="dram", bufs=1, space="DRAM") as dram,
        ):
            # ---- constants ----
            ident = cpool.tile([P, P], bf16)
            nc.sync.dma_start(ident[:], t_ident)
            iota = cpool.tile([P, P], bf16)
            nc.sync.dma_start(iota[:], t_iota)
            idx_lo = cpool.tile([P, SLP // 16], i16)
            nc.sync.dma_start(idx_lo[:], t_idx_lo)
            idx_hi = cpool.tile([P, SHP // 16], i16)
            nc.sync.dma_start(idx_hi[:], t_idx_hi)
            ier_lo = cpool.tile([P, SLP // 16], i16)
            nc.sync.dma_start(ier_lo[:], t_ier_lo)
            ier_hi = cpool.tile([P, SHP // 16], i16)
            nc.sync.dma_start(ier_hi[:], t_ier_hi)
            dr_lo = cpool.tile([P, SLP // P], bf16)
            nc.sync.dma_start(dr_lo[:], t_dr_lo)
            dr_hi = cpool.tile([P, SHP // P], bf16)
            nc.sync.dma_start(dr_hi[:], t_dr_hi)
            Ws, Bs = [], []
            for li, (ind, H, D, _) in enumerate(LAYERS):
                hd = H * D
                w = cpool.tile([ind, hd + 2 * H], bf16, tag=f"W{li}")
                nc.sync.dma_start(w[:], t_W[li])
                Ws.append(w)
                bb = cpool.tile([P, hd], f32, tag=f"br{li}")
                nc.sync.dma_start(bb[:], t_b[li])
                Bs.append(bb)

            # x_own: layer-0 input, cast f32 -> bf16 during DMA (SWDGE)
            x_own = bigpool.tile([P, xj * 128], bf16, tag="x_own")
            nc.gpsimd.dma_start(
                out=x_own[:].rearrange("p (i d) -> p i d", d=128),
                in_=t_feats.rearrange("(i p) d -> p i d", p=P))

            def _phase_a_block(li2, g, x_src_ap, tabsb2, er2):
                """Compute table row block g for layer li2 from x (bf16
                [P,128] node-major): feat|el into tabsb2, er into er2."""
                ind2, H2, D2, _ = LAYERS[li2]
                hd2 = H2 * D2
                te2 = 256 if li2 < 2 else 128
                xT_ps = psA.tile([P, P], bf16, tag="psAT")
                nc.tensor.transpose(out=xT_ps[:], in_=x_src_ap,
                                    identity=ident[:])
                xT = sb.tile([P, ind2], bf16, tag="xT")
                nc.scalar.activation(xT[:], xT_ps[:, :ind2], AF.Copy)
                f_ps = psA.tile([P, hd2 + 2 * H2], f32, tag="psA")
                nc.tensor.matmul(out=f_ps[:], lhsT=xT[:], rhs=Ws[li2][:],
                                 start=True, stop=True)
                nc.scalar.activation(
                    tabsb2[:, g * te2:g * te2 + hd2], f_ps[:, :hd2],
                    AF.Copy, scale=FP8_SCALE if li2 < 2 else 1.0)
                if li2 < 2:
                    tab_bf = tabsb2[:].bitcast(bf16)
                    nc.vector.tensor_copy(
                        tab_bf[:, g * 128 + 64:g * 128 + 64 + H2],
                        f_ps[:, hd2:hd2 + H2])
                else:
                    nc.vector.tensor_copy(
                        tabsb2[:, g * te2 + hd2:g * te2 + hd2 + H2],
                        f_ps[:, hd2:hd2 + H2])
                nc.vector.tensor_copy(er2[:, g * H2:(g + 1) * H2],
                                      f_ps[:, hd2 + H2:hd2 + 2 * H2])

            def _tab_alloc(li2):
                te2 = 256 if li2 < 2 else 128
                tdt2 = fp8 if li2 < 2 else bf16
                t2 = bigpool.tile([P, xj * te2], tdt2,
                                  tag=f"tabsb{li2 % 2}", name=f"tabsb{li2}")
                e2 = bigpool.tile([P, xj * LAYERS[li2][1]], bf16,
                                  tag=f"erown{li2 % 2}", name=f"erown{li2}")
                return dict(tab=t2, er=e2, te=te2, dt=tdt2)

            # -------- layer-0 phase A (from input features) --------
            cur_tt = _tab_alloc(0)
            for i in range(xj):
                _phase_a_block(0, i, x_own[:, i * 128:(i + 1) * 128],
                               cur_tt["tab"], cur_tt["er"])

            for li, (ind, H, D, apply_relu) in enumerate(LAYERS):
                hd = H * D
                is_fp8 = li < 2
                SCALE = FP8_SCALE if is_fp8 else 1.0
                tdt = fp8 if is_fp8 else bf16
                telem = 256 if is_fp8 else 128   # gather elem (elements)
                # next layer's table tiles, filled by the fused phase A
                # inside this layer's epilogues
                if li < 2:
                    next_tt = _tab_alloc(li + 1)

                # er table: bf16 rows of 128 (256B), er in first H cols
                er_own_d = dram.tile([NPCP, 128], bf16, tag=f"er_own{li}")
                nc.sync.dma_start(
                    er_own_d[:, :H].rearrange("(i p) h -> p i h", p=P),
                    cur_tt["er"][:].rearrange("p (i h) -> p i h", h=H))

                # ---------------- all-gather + er prefetch ----------------
                def _chunk_bounds(k):
                    g0 = k * SPAN
                    g1 = min(g0 + SPAN, xj)
                    return (g0, g1, int(lo_base[g0]), int(lo_base[g1]),
                            int(hi_base[g0]), int(hi_base[g1]))

                def _issue_er(k, er_own_d=er_own_d, _cb=_chunk_bounds):
                    _, _, lt0, lt1, ht0, ht1 = _cb(k)
                    erg_lo = erg_hi = None
                    if lt1 > lt0:
                        erg_lo = erpool.tile([P, MCL * 128], bf16,
                                             tag="erg_lo")
                        _gather(erg_lo[:, :(lt1 - lt0) * 128]
                                .rearrange("p (j d) -> p j d", d=128),
                                er_own_d[:], ier_lo[:, lt0 * 8:lt1 * 8],
                                (lt1 - lt0) * P, 128)
                    if ht1 > ht0:
                        erg_hi = erpool.tile([P, MCH * 128], bf16,
                                             tag="erg_hi")
                        _gather(erg_hi[:, :(ht1 - ht0) * 128]
                                .rearrange("p (j d) -> p j d", d=128),
                                er_own_d[:], ier_hi[:, ht0 * 8:ht1 * 8],
                                (ht1 - ht0) * P, 128)
                    return erg_lo, erg_hi

                # er gathers don't depend on the AllGather — issue the
                # first ER_LA chunks' worth, then the table DMA + AG; the er
                # drains overlap the AG's blocking wait.
                er_tiles = {}
                for k in range(min(ER_LA, nchunk)):
                    er_tiles[k] = _issue_er(k)

                tab_own_d = dram.tile([NPCP, telem], tdt, tag=f"tab_own{li}")
                nc.sync.dma_start(
                    tab_own_d[:].rearrange("(i p) d -> p i d", p=P),
                    cur_tt["tab"][:].rearrange("p (i d) -> p i d", d=telem))
                tab_full = dram.tile([n_nodes, telem], tdt,
                                     addr_space="Shared", tag=f"tab_full{li}")
                nc.gpsimd.collective_compute(
                    "AllGather", mybir.AluOpType.bypass,
                    replica_groups=[list(range(n_cores))],
                    ins=[tab_own_d[:npc, :]],
                    outs=[tab_full[:]],
                )

                def _issue_feat(k, tab_full=tab_full, telem=telem, tdt=tdt,
                                _cb=_chunk_bounds):
                    _, _, lt0, lt1, ht0, ht1 = _cb(k)
                    fgt_lo = fgt_hi = None
                    if lt1 > lt0:
                        fgt_lo = fgpool.tile([P, MCL * telem], tdt,
                                             tag="fgt_lo")
                        _gather(fgt_lo[:, :(lt1 - lt0) * telem]
                                .rearrange("p (j d) -> p j d", d=telem),
                                tab_full[:SPLIT, :] if n_nodes > SPLIT
                                else tab_full[:],
                                idx_lo[:, lt0 * 8:lt1 * 8],
                                (lt1 - lt0) * P, telem)
                    if ht1 > ht0:
                        fgt_hi = fgpool.tile([P, MCH * telem], tdt,
                                             tag="fgt_hi")
                        _gather(fgt_hi[:, :(ht1 - ht0) * telem]
                                .rearrange("p (j d) -> p j d", d=telem),
                                tab_full[SPLIT:, :],
                                idx_hi[:, ht0 * 8:ht1 * 8],
                                (ht1 - ht0) * P, telem)
                    return fgt_lo, fgt_hi

                # ---------------- edge phase ----------------
                fg_tiles = {}
                for k in range(nchunk):
                    g0, g1, lt0, lt1, ht0, ht1 = _chunk_bounds(k)
                    if k + ER_LA - 1 < nchunk and \
                            k + ER_LA - 1 not in er_tiles:
                        er_tiles[k + ER_LA - 1] = _issue_er(k + ER_LA - 1)
                    for kk in range(k, min(k + FG_LA, nchunk)):
                        if kk not in fg_tiles:
                            fg_tiles[kk] = _issue_feat(kk)
                    erg_lo, erg_hi = er_tiles.pop(k)
                    fgt_lo, fgt_hi = fg_tiles.pop(k)

                    for g in range(g0, g1):
                        nt_tot = (int(lo_base[g + 1]) - int(lo_base[g]) +
                                  int(hi_base[g + 1]) - int(hi_base[g]))
                        ng = min(npc - g * P, P)
                        if nt_tot == 0:
                            ot = sb.tile([P, hd], f32, tag="ot")
                            nc.vector.tensor_copy(ot[:], Bs[li][:, :hd])
                        else:
                            ps = pspool.tile([P, hd + H], f32, tag="ps")
                            first = True
                            done = 0
                            for zn, zb0, zb1, zdr, fgt, erg, ct0 in (
                                ("lo", int(lo_base[g]), int(lo_base[g + 1]),
                                 dr_lo, fgt_lo, erg_lo, lt0),
                                ("hi", int(hi_base[g]), int(hi_base[g + 1]),
                                 dr_hi, fgt_hi, erg_hi, ht0),
                            ):
                                ntg = zb1 - zb0
                                if ntg == 0:
                                    continue
                                rel = zb0 - ct0
                                if is_fp8:
                                    fg_bf = fgt[:].bitcast(bf16)
                                    el_ap = fg_bf.rearrange(
                                        "p (j d) -> p j d",
                                        d=128)[:, rel:rel + ntg, 64:64 + H]
                                    feat_ap = fgt[:].rearrange(
                                        "p (j d) -> p j d",
                                        d=telem)[:, rel:rel + ntg, :hd] \
                                        .rearrange("p t (h d) -> p t h d",
                                                   d=D)
                                else:
                                    el_ap = fgt[:].rearrange(
                                        "p (j d) -> p j d",
                                        d=telem)[:, rel:rel + ntg,
                                                 hd:hd + H]
                                    feat_ap = fgt[:].rearrange(
                                        "p (j d) -> p j d",
                                        d=telem)[:, rel:rel + ntg, :hd] \
                                        .rearrange("p t (h d) -> p t h d",
                                                   d=D)
                                esum = wpool.tile([P, MG * H], f32,
                                                  tag="esum")
                                nc.vector.tensor_tensor(
                                    out=esum[:, :ntg * H]
                                        .rearrange("p (j h) -> p j h", h=H),
                                    in0=el_ap,
                                    in1=erg[:].rearrange(
                                        "p (j d) -> p j d",
                                        d=128)[:, rel:rel + ntg, :H],
                                    op=OP.add)
                                # w = exp(leaky_relu(s)) =
                                #     max(exp(s), exp(.2 s)) — exps on the
                                # idle scalar engine, only the max on DVE.
                                e1 = wpool.tile([P, MG * H], f32, tag="e1")
                                nc.scalar.activation(
                                    e1[:, :ntg * H], esum[:, :ntg * H],
                                    AF.Exp)
                                e2 = wpool.tile([P, MG * H], f32, tag="e2")
                                nc.scalar.activation(
                                    e2[:, :ntg * H], esum[:, :ntg * H],
                                    AF.Exp, scale=NEG_SLOPE)
                                rhs = wpool.tile([P, MG * (hd + H)], bf16,
                                                 tag="rhs")
                                rhs3 = rhs[:, :ntg * (hd + H)].rearrange(
                                    "p (t d) -> p t d", d=hd + H)
                                w_ap = rhs3[:, :, hd:hd + H]
                                nc.vector.tensor_tensor(
                                    out=w_ap,
                                    in0=e1[:, :ntg * H].rearrange(
                                        "p (t h) -> p t h", h=H),
                                    in1=e2[:, :ntg * H].rearrange(
                                        "p (t h) -> p t h", h=H),
                                    op=OP.max)
                                nc.vector.tensor_tensor(
                                    out=rhs3[:, :, :hd].rearrange(
                                        "p t (h d) -> p t h d", d=D),
                                    in0=feat_ap,
                                    in1=w_ap.to_broadcast([P, ntg, H, D]),
                                    op=OP.mult)
                                oh = sb.tile([P, MG * P], bf16, tag="oh")
                                nc.vector.tensor_tensor(
                                    out=oh[:, :ntg * P]
                                        .rearrange("p (t d) -> p t d", d=P),
                                    in0=_midb(iota[:], ntg),
                                    in1=zdr[:, zb0:zb1]
                                        .to_broadcast([P, ntg, P]),
                                    op=OP.is_equal)
                                for t in range(ntg):
                                    done += 1
                                    nc.tensor.matmul(
                                        out=ps[:],
                                        lhsT=oh[:, t * P:(t + 1) * P],
                                        rhs=rhs[:, t * (hd + H):
                                                (t + 1) * (hd + H)],
                                        start=first, stop=(done == nt_tot))
                                    first = False
                            # epilogue: divide (incl 1/SCALE), bias
                            den = sb.tile([P, H], f32, tag="den")
                            nc.vector.tensor_scalar_max(
                                den[:], ps[:, hd:hd + H], 1e-12)
                            rec = sb.tile([P, H], f32, tag="rec")
                            nc.vector.reciprocal(rec[:], den[:])
                            if SCALE != 1.0:
                                nc.vector.tensor_scalar_mul(
                                    rec[:], rec[:], 1.0 / SCALE)
                            ot = sb.tile([P, hd], f32, tag="ot")
                            nc.vector.tensor_tensor(
                                out=ot[:].rearrange("p (h d) -> p h d", d=D),
                                in0=ps[:, :hd].rearrange(
                                    "p (h d) -> p h d", d=D),
                                in1=rec[:].to_broadcast([P, H, D]),
                                op=OP.mult)
                            nc.vector.tensor_tensor(
                                out=ot[:], in0=ot[:], in1=Bs[li][:, :hd],
                                op=OP.add)
                        if li < 2:
                            # relu -> bf16 x block, feed next layer's phase A
                            # in SBUF (no DRAM round-trip)
                            x_blk = sb.tile([P, hd], bf16, tag="x_blk")
                            nc.vector.tensor_scalar_max(x_blk[:], ot[:], 0.0)
                            _phase_a_block(li + 1, g, x_blk[:],
                                           next_tt["tab"], next_tt["er"])
                        else:
                            nc.sync.dma_start(t_out[g * P:g * P + ng, :],
                                              ot[:ng, :])
                if li < 2:
                    cur_tt = next_tt
    nc.compile()
    return nc


# ============================ entry point ====================================

def _meta_key(meta):
    return (meta["TL"], meta["TH"], meta["SL"], meta["SH"],
            meta["xj"], meta["npc"], meta["n_nodes"], meta["n_cores"])


def _get_compiled(meta):
    key = _meta_key(meta)
    if key not in _cache:
        _cache[key] = _build_program(meta)
    return _cache[key]


def _make_in_maps(inputs, meta, per_core):
    import ml_dtypes
    bf = ml_dtypes.bfloat16
    f32 = np.float32
    xj, npc = meta["xj"], meta["npc"]
    n_cores = meta["n_cores"]
    iota_rep = np.tile(np.arange(P, dtype=f32).astype(bf), (P, 1))
    ident = np.eye(P, dtype=f32).astype(bf)
    common = {"iota_rep": iota_rep, "identity_bf": ident}
    for li in range(len(LAYERS)):
        ind, H, D, _ = LAYERS[li]
        W = np.asarray(inputs[f"W{li}"], f32)
        al = np.asarray(inputs[f"al{li}"], f32)
        ar = np.asarray(inputs[f"ar{li}"], f32)
        b = np.asarray(inputs[f"b{li}"], f32)
        hd = H * D
        al_flat = np.zeros((hd, H), f32)
        ar_flat = np.zeros((hd, H), f32)
        for h in range(H):
            al_flat[h * D:(h + 1) * D, h] = al[h]
            ar_flat[h * D:(h + 1) * D, h] = ar[h]
        wcat = np.concatenate([W, W @ al_flat, W @ ar_flat], axis=1)
        common[f"Wcat{li}"] = wcat.astype(bf)
        common[f"br{li}"] = np.tile(b[None, :], (P, 1)).astype(f32)

    feats = np.asarray(inputs["features"], f32)
    in_maps = []
    for c in range(n_cores):
        pc = per_core[c]
        fo = np.zeros((xj * P, 128), f32)
        fo[:npc] = feats[c * npc:(c + 1) * npc]
        in_maps.append({
            **common,
            "features_own": fo,
            "idx_lo": pc["idx_lo"], "idx_hi": pc["idx_hi"],
            "idx_er_lo": pc["idx_er_lo"], "idx_er_hi": pc["idx_er_hi"],
            "dr_lo": pc["dr_lo"], "dr_hi": pc["dr_hi"],
        })
    return in_maps


def kernel(**inputs):
    from concourse import bass_utils

    src = np.asarray(inputs["src"]).astype(np.int64)
    dst = np.asarray(inputs["dst"]).astype(np.int64)
    n_nodes = np.asarray(inputs["features"]).shape[0]
    meta, per_core = _preprocess(src, dst, n_nodes, N_CORES)
    nc = _get_compiled(meta)
    in_maps = _make_in_maps(inputs, meta, per_core)
    n_cores = meta["n_cores"]
    res = bass_utils.run_bass_kernel_spmd(
        nc, in_maps, core_ids=list(range(n_cores)),
        trace=bool(last_run_info.get("trace", False)))
    last_run_info["exec_time_ns"] = res.exec_time_ns
    last_run_info["profile_json"] = res.profile_json
    last_run_info["res"] = res

    npc = meta["npc"]
    out = np.empty((n_nodes, OUT_DIM), np.float32)
    for c in range(n_cores):
        out[c * npc:(c + 1) * npc] = res.results[c]["out"]
    return out
